# revision 11
# baseline (speedup 1.0000x reference)
"""Trainium2 Bass kernel for CausalSelfAttention (RoPE + GQA), 8-core SPMD.

Sharding: 8 cores = 4 batches x 2 query-halves. Each core owns four
query-256-blocks paired {i, 7-i} so causal work is balanced. Keys are
PERMUTED per core: block order = [own q-blocks (desc causal depth), then
remaining blocks ascending]. Slot s consumes the static key-chunk range
[2s, 2s+PAD_s); its diagonal chunks 2s..2s+1 are emitted last so one bf16
mask multiply per (head, slot) covers diag+pad. The first 1024 key columns
ARE the core's queries, so K rope tables double as Q tables. Every core
runs an identical instruction stream; all variation is input data.

Single fused pipeline per core (vs. baseline's 3 serial phases):
  KV windows are processed in the order [768:1280), [1280:1792),
  [256:768), [0:256), [1792:2048) so attention slot 3 (key chunks 6-9)
  unblocks after ONE window; Q projections are emitted per-slot. RoPE
  needs q/k projected with pair-swapped weights too -- instead of a second
  full projection (5 matmuls), the PSUM is staged to SBUF as bf16 once
  (Act copy, idle engine) and pair-swapped by a single 128x128
  permutation matmul (1 matmul). Post-projection tensors (kt/qth/v/p/ypr,
  tables, masks) are bf16: matmuls stay 1 cycle/row at any width and the
  mask multiply + rope adds hit the DVE 4x mode. Attention per (head,
  slot): S = K^T.T@Q^T with keys on partitions, exp on ScalarE
  (PSUM->bf16, scale=1/8), one bf16 mask multiply, P.V as bf16 matmul
  with a ones-augmented V column yielding the softmax denominator free,
  reciprocal + gpsimd partition-broadcast divide. Attention heads are
  interleaved with projection/output-projection chunks in emission order
  to keep PE busy while ScalarE chews exps; out-projection is emitted in
  256-column slot pieces so only the last slot's piece trails the
  attention stream. DMAs are issued in need-order; x is loaded once and
  stays in SBUF.
"""
import sys

sys.path.insert(0, "/opt/trn_rl_repo")

import numpy as np
import ml_dtypes

B, T, C = 4, 2048, 576
H, HKV, D = 9, 3, 64
THETA = 10000.0
QB = 256                      # query block
TQ = 1024                     # queries per core
SLOT_PAD = [16, 12, 8, 4]     # padded key-chunk counts per slot
QBLOCKS = [[7, 5, 2, 0], [6, 4, 3, 1]]   # q-256-block ids per half j
KEYORDER = [[7, 5, 2, 0, 1, 3, 4, 6], [6, 4, 3, 1, 0, 2, 5, 7]]
CCX = [(0, 128), (128, 128), (256, 128), (384, 128), (512, 65)]   # x chunks (577 rows incl ones)
CCQ = [(0, 128), (128, 128), (256, 128), (384, 128), (512, 64)]   # 576-row chunks
MM = [(0, 128), (128, 128), (256, 128), (384, 128), (512, 64)]    # output-dim chunks of 576
# kv-window processing order (col0, width): slot3's chunks 6..9 first
KVWINS = [(768, 512), (1280, 512), (256, 512), (0, 256), (1792, 256)]
# q-window processing order (col0, width): slot 3, slot 2, slots 0+1
QWINS = [(768, 256), (512, 256), (0, 512)]


def _slot_seq(s):
    """Key-chunk emission order for slot s: fulls, then the two diag chunks."""
    return list(range(2 * s + 2, 2 * s + SLOT_PAD[s])) + [2 * s, 2 * s + 1]


_PROG = None


def _rne12(x):
    """Round fp32 to f32r (RNE, drop 12 mantissa bits) -- matches TRN2."""
    b = np.ascontiguousarray(x, np.float32).view(np.uint32).astype(np.uint64)
    lsb = (b >> np.uint64(12)) & np.uint64(1)
    r = (b + np.uint64(2047) + lsb) >> np.uint64(12) << np.uint64(12)
    return (r & np.uint64(0xFFFFFFFF)).astype(np.uint32).view(np.float32)


def _build_program():
    import concourse.bacc as bacc
    import concourse.mybir as mybir
    import concourse.tile as tile

    dt = mybir.dt
    f32, f32r, bf16 = dt.float32, dt.float32r, dt.bfloat16
    AF = mybir.ActivationFunctionType

    nc = bacc.Bacc("TRN2", target_bir_lowering=False, debug=False, num_devices=8)

    def inp(name, shape, d=f32):
        return nc.declare_dram_parameter(name, shape, d, isOutput=False)

    xkT = inp("xkT", [577, T], f32r)
    wqT = inp("wqT", [C, C], f32r)
    wkT = inp("wkT", [C, HKV * D], f32r)
    wvT = inp("wvT", [577, 260], f32r)
    woT = inp("woT", [C, C], bf16)
    c2k = inp("c2k", [128, T], bf16)
    s2k = inp("s2k", [128, T], f32)
    masksp = inp("masks", [16 * 128, QB], bf16)
    permp = inp("perm", [128, 128], bf16)
    yT = nc.declare_dram_parameter("yT", [C, TQ], f32, isOutput=True)

    with tile.TileContext(nc) as tc:
        with (
            tc.tile_pool(name="const", bufs=1) as cp,
            tc.tile_pool(name="rope", bufs=2) as rp,
            tc.tile_pool(name="sbst", bufs=2) as sbp,
            tc.tile_pool(name="pwork", bufs=3) as pw,
            tc.tile_pool(name="ost", bufs=2) as osp,
            tc.tile_pool(name="dvw", bufs=2) as dvp,
            tc.tile_pool(name="psP", bufs=2, space="PSUM") as psP,
            tc.tile_pool(name="psW", bufs=1, space="PSUM") as psW,
            tc.tile_pool(name="psS", bufs=2, space="PSUM") as psS,
            tc.tile_pool(name="psY", bufs=1, space="PSUM") as psYp,
        ):
            # ---------------- persistent SBUF tiles --------------------
            x_r = [cp.tile([128, T], f32r, tag=f"x{i}", name=f"x{i}")
                   for i in range(5)]
            kt_h = [cp.tile([64, T], bf16, tag=f"kt{g}", name=f"kt{g}")
                    for g in range(HKV)]
            qth = [cp.tile([64, TQ], bf16, tag=f"qth{h}", name=f"qth{h}")
                   for h in range(H)]
            v_t = [cp.tile([128, 260], bf16, tag=f"v{c}", name=f"v{c}")
                   for c in range(16)]
            ypr = [cp.tile([128, TQ], bf16, tag=f"ypr{p}", name=f"ypr{p}")
                   for p in range(5)]
            wq_r = [cp.tile([128, C], f32r, tag=f"wq{i}", name=f"wq{i}")
                    for i in range(5)]
            wk_r = [cp.tile([128, HKV * D], f32r, tag=f"wk{i}", name=f"wk{i}")
                    for i in range(5)]
            wv_r = [cp.tile([128, 260], f32r, tag=f"wv{i}", name=f"wv{i}")
                    for i in range(5)]
            wo_r = [cp.tile([128, C], bf16, tag=f"wo{i}", name=f"wo{i}")
                    for i in range(5)]
            m_b = cp.tile([128, 16 * QB], bf16, tag="masks", name="masks")
            c2k_t = cp.tile([128, T], bf16, tag="c2k", name="c2k")
            s2k_t = cp.tile([128, T], f32, tag="s2k", name="s2k")
            perm_t = cp.tile([128, 128], bf16, tag="perm", name="perm")
            y_all = psYp.tile([65, 512], f32, tag="yall", name="yall")

            # ---------------- DMAs in need-order -----------------------
            def dma_w(tiles, param, chunks, cols):
                for i, (k0, kl) in enumerate(chunks):
                    nc.sync.dma_start(tiles[i][:kl, :], param[k0:k0 + kl, :])

            def dma_x(c0, w):
                for i, (k0, kl) in enumerate(CCX):
                    nc.sync.dma_start(x_r[i][:kl, c0:c0 + w],
                                      xkT[k0:k0 + kl, c0:c0 + w])

            dma_w(wk_r, wkT, CCQ, HKV * D)
            dma_w(wv_r, wvT, CCX, 260)
            dma_x(768, 512)                       # kv window A / q slot 3
            nc.sync.dma_start(c2k_t[:], c2k[:])
            nc.sync.dma_start(s2k_t[:], s2k[:])
            nc.sync.dma_start(perm_t[:], permp[:])
            dma_w(wq_r, wqT, CCQ, C)
            dma_x(1280, 512)                      # kv window B
            dma_x(512, 256)                       # q slot 2
            dma_x(256, 256)                       # rest of kv window C
            dma_x(0, 256)                         # kv window D / q slots 0,1
            for i in range(16):
                nc.sync.dma_start(m_b[:, i * QB:(i + 1) * QB],
                                  masksp[i * 128:(i + 1) * 128, :])
            dma_x(1792, 256)                      # kv window E
            dma_w(wo_r, woT, MM, C)

            # ---------------- rope combine -----------------------------
            def rope(ps, sb_t, mrows, c0, w, dsts):
                """dsts[bi][:, c0:c0+w] = sb*cos + perm(sb)*sin tables."""
                pw_ = psW.tile([128, 512], f32, tag="psw", name="psw")
                nc.tensor.matmul(pw_[:mrows, :w], perm_t[:mrows, :mrows],
                                 sb_t[:mrows, :w], start=True, stop=True)
                t1 = rp.tile([128, 512], bf16, tag="rope1", name="rope1")
                t2 = rp.tile([128, 512], bf16, tag="rope2", name="rope2")
                nc.vector.tensor_mul(t1[:mrows, :w], sb_t[:mrows, :w],
                                     c2k_t[:mrows, c0:c0 + w])
                nc.vector.tensor_mul(t2[:mrows, :w], pw_[:mrows, :w],
                                     s2k_t[:mrows, c0:c0 + w])
                for bi, dt_ in enumerate(dsts):
                    nc.vector.tensor_add(dt_[0:64, c0:c0 + w],
                                         t1[64 * bi:64 * bi + 64, :w],
                                         t2[64 * bi:64 * bi + 64, :w])

            # ---------------- projection thunks ------------------------
            def kproj_chunk(c0, w, mi):
                mc0, mrows = (0, 128) if mi == 0 else (128, 64)
                dsts = [kt_h[0], kt_h[1]] if mi == 0 else [kt_h[2]]

                def thunk():
                    ps = psP.tile([128, 512], f32, tag="pj", name="pj")
                    for ci, (k0, kl) in enumerate(CCQ):
                        nc.tensor.matmul(ps[:mrows, :w],
                                         wk_r[ci][:kl, mc0:mc0 + mrows],
                                         x_r[ci][:kl, c0:c0 + w],
                                         start=(ci == 0), stop=(ci == 4))
                    sb_t = sbp.tile([128, 512], bf16, tag="sb", name="sb")
                    nc.vector.tensor_copy(sb_t[:mrows, :w], ps[:mrows, :w])
                    rope(ps, sb_t, mrows, c0, w, dsts)
                return thunk

            def qproj_chunk(c0, w, m):
                mc0, mrows = MM[m]
                dsts = [qth[2 * m], qth[2 * m + 1]] if m < 4 else [qth[8]]

                def thunk():
                    ps = psP.tile([128, 512], f32, tag="pj", name="pj")
                    for ci, (k0, kl) in enumerate(CCQ):
                        nc.tensor.matmul(ps[:mrows, :w],
                                         wq_r[ci][:kl, mc0:mc0 + mrows],
                                         x_r[ci][:kl, c0:c0 + w],
                                         start=(ci == 0), stop=(ci == 4))
                    sb_t = sbp.tile([128, 512], bf16, tag="sb", name="sb")
                    nc.vector.tensor_copy(sb_t[:mrows, :w], ps[:mrows, :w])
                    rope(ps, sb_t, mrows, c0, w, dsts)
                return thunk

            def vproj_chunk(c):
                def thunk():
                    ps = psP.tile([128, 512], f32, tag="pj", name="pj")
                    for ci, (k0, kl) in enumerate(CCX):
                        nc.tensor.matmul(ps[:, :260],
                                         x_r[ci][:kl, 128 * c:128 * (c + 1)],
                                         wv_r[ci][:kl, :],
                                         start=(ci == 0), stop=(ci == 4))
                    nc.scalar.activation(v_t[c][:], ps[:, :260], AF.Copy)
                return thunk

            def kv_thunks(c0, w):
                th = [kproj_chunk(c0, w, 0), kproj_chunk(c0, w, 1)]
                th += [vproj_chunk(c0 // 128 + ti) for ti in range(w // 128)]
                return th

            def oproj_thunks(s):
                def piece(m):
                    mc0, mrows = MM[m]

                    def thunk():
                        ps = psP.tile([128, 512], f32, tag="pj", name="pj")
                        for p, (pc0, pl) in enumerate(MM):
                            nc.tensor.matmul(
                                ps[:mrows, :QB],
                                wo_r[p][:pl, mc0:mc0 + mrows],
                                ypr[p][:pl, QB * s:QB * (s + 1)],
                                start=(p == 0), stop=(p == 4))
                        ost = osp.tile([128, QB], f32, tag="ost", name="ost")
                        nc.vector.tensor_copy(ost[:mrows, :], ps[:mrows, :QB])
                        nc.sync.dma_start(
                            yT[mc0:mc0 + mrows, QB * s:QB * (s + 1)],
                            ost[:mrows, :])
                    return thunk
                return [piece(m) for m in range(5)]

            # ---------------- attention --------------------------------
            ycnt = [0]

            def attn_head(s, h):
                seq = _slot_seq(s)
                n = len(seq)
                g = h // 3
                hp, hr = h // 2, 64 * (h % 2)
                yo = QB * (ycnt[0] % 2)
                ycnt[0] += 1
                for sc in range(n // 4):
                    sp = psS.tile([128, 4 * QB], f32, tag="scores",
                                  name="scores")
                    for i in range(4):
                        c = seq[4 * sc + i]
                        nc.tensor.matmul(
                            sp[:, QB * i:QB * (i + 1)],
                            kt_h[g][0:64, 128 * c:128 * (c + 1)],
                            qth[h][0:64, QB * s:QB * (s + 1)],
                            start=True, stop=True)
                    p_b = pw.tile([128, 4 * QB], bf16, tag="p", name="p")
                    nc.scalar.activation(p_b[:], sp[:], AF.Exp, scale=0.125)
                    if sc == n // 4 - 1:
                        nc.vector.tensor_mul(
                            p_b[:], p_b[:], m_b[:, 1024 * s:1024 * (s + 1)])
                    for i in range(4):
                        c = seq[4 * sc + i]
                        nc.tensor.matmul(
                            y_all[:, yo:yo + QB],
                            v_t[c][:, 65 * g:65 * g + 65],
                            p_b[:, QB * i:QB * (i + 1)],
                            start=(4 * sc + i == 0),
                            stop=(4 * sc + i == n - 1))
                recip = dvp.tile([1, QB], f32, tag="recip", name="recip")
                nc.vector.reciprocal(recip[:], y_all[64:65, yo:yo + QB])
                rb_sb = dvp.tile([D, QB], f32, tag="rb", name="rb")
                nc.gpsimd.partition_broadcast(rb_sb[:], recip[:], D)
                nc.vector.tensor_mul(
                    ypr[hp][hr:hr + 64, QB * s:QB * (s + 1)],
                    y_all[0:64, yo:yo + QB], rb_sb[:])

            def interleave(s, proj):
                """Emit slot s's 9 heads, spreading proj thunks between."""
                j = 0
                for i in range(H):
                    attn_head(s, i)
                    want = (i + 1) * len(proj) // H
                    while j < want:
                        proj[j]()
                        j += 1

            # ---------------- emission schedule ------------------------
            for t in kv_thunks(768, 512):        # window A: chunks 6..9
                t()
            for m in range(5):                   # q slot 3
                qproj_chunk(768, 256, m)()

            kvC = kv_thunks(256, 512)                # [K-m0, K-m1, V2..V5]
            # slot 2 consumes kvC's kt cols + diag v chunks 4,5 -- those
            # must be emitted before slot 2's heads (Tile deps follow
            # emission order), so they ride in slot 3's stream.
            interleave(3, kv_thunks(1280, 512)       # window B: 10..13
                       + [qproj_chunk(512, 256, m) for m in range(5)]
                       + [kvC[0], kvC[1], kvC[4], kvC[5]])
            interleave(2, [kvC[2], kvC[3]]           # v chunks 2,3 (slot 1)
                       + [qproj_chunk(0, 512, m) for m in range(5)])
            interleave(1, kv_thunks(0, 256)          # window D: 0..1
                       + kv_thunks(1792, 256)        # window E: 14..15
                       + oproj_thunks(2) + oproj_thunks(3))
            interleave(0, oproj_thunks(1))
            for t in oproj_thunks(0):
                t()

    nc.compile()
    return nc


def _get_program():
    global _PROG
    if _PROG is None:
        _PROG = _build_program()
    return _PROG


def _neox_perm(nheads, swap=False):
    p = []
    for h in range(nheads):
        ev = [64 * h + 2 * j for j in range(32)]
        od = [64 * h + 2 * j + 1 for j in range(32)]
        p += (od + ev) if swap else (ev + od)
    return np.array(p)


_CONSTS = None


def _static_consts():
    """Input-independent per-core constants (tables, masks, key orders)."""
    global _CONSTS
    if _CONSTS is not None:
        return _CONSTS
    invf = THETA ** (-np.arange(32, dtype=np.float64) / 32)

    def tables(pos):
        ang = pos[None, :] * invf[:, None]
        cos, sin = np.cos(ang), np.sin(ang)
        c2 = np.tile(cos, (4, 1)).astype(ml_dtypes.bfloat16)
        s2 = np.tile(np.vstack([-sin, sin]), (2, 1)).astype(np.float32)
        return c2, s2

    per_j = []
    for j in range(2):
        keypos = np.concatenate(
            [np.arange(QB * q, QB * (q + 1)) for q in KEYORDER[j]])
        qsel = keypos[:TQ]          # queries = first 1024 permuted keys
        c2k, s2k = tables(keypos.astype(np.float64))
        masks = np.zeros((16 * 128, QB), np.float32)
        for s in range(4):
            seq = _slot_seq(s)
            qpos = keypos[QB * s:QB * (s + 1)]
            for k in range(4):
                c = seq[-4 + k]
                kpos = keypos[128 * c:128 * (c + 1)]
                masks[(4 * s + k) * 128:(4 * s + k + 1) * 128] = (
                    kpos[:, None] <= qpos[None, :]).astype(np.float32)
        per_j.append((keypos, qsel, c2k, s2k,
                      masks.astype(ml_dtypes.bfloat16)))
    _CONSTS = per_j
    return _CONSTS


_PERM = None


def _swap_perm():
    """[128,128] bf16: P[k,m]=1 iff k = pair-swap(m) (+-32 within 64-blocks)."""
    global _PERM
    if _PERM is None:
        P = np.zeros((128, 128), np.float32)
        m = np.arange(128)
        k = np.where(m % 64 < 32, m + 32, m - 32)
        P[k, m] = 1.0
        _PERM = P.astype(ml_dtypes.bfloat16)
    return _PERM


def _host_prep(x, Wq, Wk, Wv, Wo):
    wqT = _rne12(Wq[_neox_perm(H)].T)
    wkT = _rne12(Wk[_neox_perm(HKV)].T)
    woT = Wo.T.astype(ml_dtypes.bfloat16)
    wvT = np.zeros((577, 260), np.float32)
    for g in range(HKV):
        wvT[:C, 65 * g:65 * g + 64] = Wv[64 * g:64 * g + 64].T
        wvT[576, 65 * g + 64] = 1.0
    wvT = _rne12(wvT)

    per_j = _static_consts()
    perm = _swap_perm()
    x = _rne12(x)
    ones = np.ones((1, T), np.float32)
    in_maps = []
    core_meta = []
    for b in range(B):
        xbT = x[b].T
        for j in range(2):
            keypos, qsel, c2k, s2k, masks = per_j[j]
            xkT = np.vstack([xbT[:, keypos], ones])
            in_maps.append({
                "xkT": xkT,
                "wqT": wqT, "wkT": wkT, "wvT": wvT, "woT": woT,
                "c2k": c2k, "s2k": s2k,
                "masks": masks, "perm": perm,
            })
            core_meta.append((b, qsel))
    return in_maps, core_meta


def kernel(x, Wq, Wk, Wv, Wo):
    x = np.asarray(x, np.float32)
    Wq = np.asarray(Wq, np.float32)
    Wk = np.asarray(Wk, np.float32)
    Wv = np.asarray(Wv, np.float32)
    Wo = np.asarray(Wo, np.float32)

    from concourse.bass_utils import run_bass_kernel_spmd

    nc = _get_program()
    in_maps, core_meta = _host_prep(x, Wq, Wk, Wv, Wo)
    res = run_bass_kernel_spmd(nc, in_maps, list(range(8)))

    out = np.empty((B, T, C), np.float32)
    for core, (b, qsel) in enumerate(core_meta):
        out[b, qsel, :] = res.results[core]["yT"].T
    return out


# revision 21
# speedup vs baseline: 1.0812x; 1.0812x over previous
"""Trainium2 Bass kernel for CausalSelfAttention (RoPE + GQA), 8-core SPMD.

Sharding: 8 cores = 4 batches x 2 query-halves. Each core owns four
query-256-blocks paired {i, 7-i} so causal work is balanced. Keys are
PERMUTED per core: block order = [own q-blocks (desc causal depth), then
remaining blocks ascending]. Slot s consumes the static key-chunk range
[2s, 2s+PAD_s); its diagonal chunks 2s..2s+1 are emitted last so one bf16
mask multiply per (head, slot) covers diag+pad. The first 1024 key columns
ARE the core's queries, so K rope tables double as Q tables. Every core
runs an identical instruction stream; all variation is input data.

Single fused pipeline per core (vs. baseline's 3 serial phases):
  KV windows are processed in the order [768:1280), [1280:1792),
  [256:768), [0:256), [1792:2048) so attention slot 3 (key chunks 6-9)
  unblocks after ONE window; Q projections are emitted per-slot. RoPE
  needs q/k projected with pair-swapped weights too -- instead of a second
  full projection (5 matmuls), the PSUM is staged to SBUF as bf16 once
  (Act copy, idle engine) and pair-swapped by a single 128x128
  permutation matmul (1 matmul). Post-projection tensors (kt/qth/v/p/ypr,
  tables, masks) are bf16: matmuls stay 1 cycle/row at any width and the
  mask multiply + rope adds hit the DVE 4x mode. Attention per (head,
  slot): S = K^T.T@Q^T with keys on partitions, exp on ScalarE
  (PSUM->bf16, scale=1/8), one bf16 mask multiply, P.V as bf16 matmul
  with a ones-augmented V column yielding the softmax denominator free,
  reciprocal + gpsimd partition-broadcast divide. Attention heads are
  interleaved with projection/output-projection chunks in emission order
  to keep PE busy while ScalarE chews exps; out-projection is emitted in
  256-column slot pieces so only the last slot's piece trails the
  attention stream. DMAs are issued in need-order; x is loaded once and
  stays in SBUF.
"""
import sys

sys.path.insert(0, "/opt/trn_rl_repo")

import numpy as np
import ml_dtypes

B, T, C = 4, 2048, 576
H, HKV, D = 9, 3, 64
THETA = 10000.0
QB = 256                      # query block
TQ = 1024                     # queries per core
SLOT_PAD = [16, 12, 8, 4]     # padded key-chunk counts per slot
QBLOCKS = [[7, 5, 2, 0], [6, 4, 3, 1]]   # q-256-block ids per half j
KEYORDER = [[7, 5, 2, 0, 1, 3, 4, 6], [6, 4, 3, 1, 0, 2, 5, 7]]
CCX = [(0, 128), (128, 128), (256, 128), (384, 128), (512, 65)]   # x chunks (577 rows incl ones)
CCQ = [(0, 128), (128, 128), (256, 128), (384, 128), (512, 64)]   # 576-row chunks
MM = [(0, 128), (128, 128), (256, 128), (384, 128), (512, 64)]    # output-dim chunks of 576
# kv-window processing order (col0, width): slot3's chunks 6..9 first
KVWINS = [(768, 512), (1280, 512), (256, 512), (0, 256), (1792, 256)]
# q-window processing order (col0, width): slot 3, slot 2, slots 0+1
QWINS = [(768, 256), (512, 256), (0, 512)]


def _slot_seq(s):
    """Key-chunk emission order for slot s: fulls, then the two diag chunks."""
    return list(range(2 * s + 2, 2 * s + SLOT_PAD[s])) + [2 * s, 2 * s + 1]


_PROG = None


def _rne12(x):
    """Round fp32 to f32r (RNE, drop 12 mantissa bits) -- matches TRN2."""
    b = np.ascontiguousarray(x, np.float32).view(np.uint32).astype(np.uint64)
    lsb = (b >> np.uint64(12)) & np.uint64(1)
    r = (b + np.uint64(2047) + lsb) >> np.uint64(12) << np.uint64(12)
    return (r & np.uint64(0xFFFFFFFF)).astype(np.uint32).view(np.float32)


def _build_program():
    import concourse.bacc as bacc
    import concourse.mybir as mybir
    import concourse.tile as tile

    dt = mybir.dt
    f32, f32r, bf16 = dt.float32, dt.float32r, dt.bfloat16
    AF = mybir.ActivationFunctionType

    nc = bacc.Bacc("TRN2", target_bir_lowering=False, debug=False, num_devices=8)

    def inp(name, shape, d=f32):
        return nc.declare_dram_parameter(name, shape, d, isOutput=False)

    xkT = inp("xkT", [577, T], bf16)
    wqT = inp("wqT", [C, C], bf16)
    wkT = inp("wkT", [C, HKV * D], bf16)
    wvT = inp("wvT", [577, 260], bf16)
    woT = inp("woT", [C, C], bf16)
    c2k = inp("c2k", [32, T], bf16)
    s2k = inp("s2k", [64, T], f32)
    masksp = inp("masks", [128, 16 * QB], bf16)
    permp = inp("perm", [128, 128], bf16)
    yT = nc.declare_dram_parameter("yT", [C, TQ], f32, isOutput=True)

    with tile.TileContext(nc) as tc:
        with (
            tc.tile_pool(name="const", bufs=1) as cp,
            tc.tile_pool(name="rope", bufs=2) as rp,
            tc.tile_pool(name="sbst", bufs=2) as sbp,
            tc.tile_pool(name="pwork", bufs=3) as pw,
            tc.tile_pool(name="ost", bufs=2) as osp,
            tc.tile_pool(name="dvw", bufs=2) as dvp,
            tc.tile_pool(name="psP", bufs=2, space="PSUM") as psP,
            tc.tile_pool(name="psS", bufs=2, space="PSUM") as psS,
            tc.tile_pool(name="psY", bufs=1, space="PSUM") as psYp,
        ):
            # ---------------- persistent SBUF tiles --------------------
            x_r = [cp.tile([128, T], bf16, tag=f"x{i}", name=f"x{i}")
                   for i in range(5)]
            kt_h = [cp.tile([64, T], bf16, tag=f"kt{g}", name=f"kt{g}")
                    for g in range(HKV)]
            qth = [cp.tile([64, TQ], bf16, tag=f"qth{h}", name=f"qth{h}")
                   for h in range(H)]
            v_t = [cp.tile([128, 260], bf16, tag=f"v{c}", name=f"v{c}")
                   for c in range(16)]
            ypr = [cp.tile([128, TQ], bf16, tag=f"ypr{p}", name=f"ypr{p}")
                   for p in range(5)]
            wq_r = [cp.tile([128, C], bf16, tag=f"wq{i}", name=f"wq{i}")
                    for i in range(5)]
            wk_r = [cp.tile([128, HKV * D], bf16, tag=f"wk{i}", name=f"wk{i}")
                    for i in range(5)]
            wv_r = [cp.tile([128, 260], bf16, tag=f"wv{i}", name=f"wv{i}")
                    for i in range(5)]
            wo_r = [cp.tile([128, C], bf16, tag=f"wo{i}", name=f"wo{i}")
                    for i in range(5)]
            m_b = cp.tile([128, 16 * QB], bf16, tag="masks", name="masks")
            c2k_t = cp.tile([128, T], bf16, tag="c2k", name="c2k")
            s2k_t = cp.tile([128, T], f32, tag="s2k", name="s2k")
            perm_t = cp.tile([128, 128], bf16, tag="perm", name="perm")
            y_all = psYp.tile([65, 1024], f32, tag="yall", name="yall")

            # ---------------- DMAs in need-order -----------------------
            def dma_w(tiles, param, chunks, cols):
                for i, (k0, kl) in enumerate(chunks):
                    nc.sync.dma_start(tiles[i][:kl, :], param[k0:k0 + kl, :])

            def dma_x(c0, w):
                for i, (k0, kl) in enumerate(CCX):
                    nc.sync.dma_start(x_r[i][:kl, c0:c0 + w],
                                      xkT[k0:k0 + kl, c0:c0 + w])

            dma_w(wk_r, wkT, CCQ, HKV * D)
            dma_x(768, 512)                       # kv window A / q slot 3
            nc.sync.dma_start(c2k_t[:32, :], c2k[:])
            nc.sync.dma_start(s2k_t[:64, :], s2k[:])
            nc.sync.dma_start(perm_t[:], permp[:])
            # tables repeat with period 32 (cos) / 64 (sin): expand on-chip
            nc.vector.tensor_copy(c2k_t[32:64, :], c2k_t[0:32, :])
            nc.vector.tensor_copy(c2k_t[64:128, :], c2k_t[0:64, :])
            nc.vector.tensor_copy(s2k_t[64:128, :], s2k_t[0:64, :])
            dma_w(wv_r, wvT, CCX, 260)
            dma_w(wq_r, wqT, CCQ, C)
            dma_x(1280, 768)                      # kv windows B + E
            dma_x(0, 768)                         # windows C, D + q slots 0-2
            nc.sync.dma_start(m_b[:], masksp[:])
            dma_w(wo_r, woT, MM, C)

            # ---------------- rope combine -----------------------------
            def rope(ps, sb_t, mrows, c0, w, dsts):
                """dsts[bi][:, c0:c0+w] = sb*cos + perm(sb)*sin tables."""
                pw_ = psP.tile([128, 512], f32, tag="pj", name="pj")
                nc.tensor.matmul(pw_[:mrows, :w], perm_t[:mrows, :mrows],
                                 sb_t[:mrows, :w], start=True, stop=True)
                t1 = rp.tile([128, 512], bf16, tag="rope1", name="rope1")
                t2 = rp.tile([128, 512], bf16, tag="rope2", name="rope2")
                nc.vector.tensor_mul(t1[:mrows, :w], sb_t[:mrows, :w],
                                     c2k_t[:mrows, c0:c0 + w])
                nc.vector.tensor_mul(t2[:mrows, :w], pw_[:mrows, :w],
                                     s2k_t[:mrows, c0:c0 + w])
                for bi, dt_ in enumerate(dsts):
                    nc.vector.tensor_add(dt_[0:64, c0:c0 + w],
                                         t1[64 * bi:64 * bi + 64, :w],
                                         t2[64 * bi:64 * bi + 64, :w])

            # ---------------- projection thunks ------------------------
            def kproj_chunk(c0, w, mi):
                mc0, mrows = (0, 128) if mi == 0 else (128, 64)
                dsts = [kt_h[0], kt_h[1]] if mi == 0 else [kt_h[2]]

                def thunk():
                    ps = psP.tile([128, 512], f32, tag="pj", name="pj")
                    for ci, (k0, kl) in enumerate(CCQ):
                        nc.tensor.matmul(ps[:mrows, :w],
                                         wk_r[ci][:kl, mc0:mc0 + mrows],
                                         x_r[ci][:kl, c0:c0 + w],
                                         start=(ci == 0), stop=(ci == 4))
                    sb_t = sbp.tile([128, 512], bf16, tag="sb", name="sb")
                    nc.scalar.activation(sb_t[:mrows, :w], ps[:mrows, :w],
                                         AF.Copy)
                    rope(ps, sb_t, mrows, c0, w, dsts)
                return thunk

            def qproj_chunk(c0, w, m):
                mc0, mrows = MM[m]
                dsts = [qth[2 * m], qth[2 * m + 1]] if m < 4 else [qth[8]]

                def thunk():
                    ps = psP.tile([128, 512], f32, tag="pj", name="pj")
                    for ci, (k0, kl) in enumerate(CCQ):
                        nc.tensor.matmul(ps[:mrows, :w],
                                         wq_r[ci][:kl, mc0:mc0 + mrows],
                                         x_r[ci][:kl, c0:c0 + w],
                                         start=(ci == 0), stop=(ci == 4))
                    sb_t = sbp.tile([128, 512], bf16, tag="sb", name="sb")
                    nc.scalar.activation(sb_t[:mrows, :w], ps[:mrows, :w],
                                         AF.Copy)
                    rope(ps, sb_t, mrows, c0, w, dsts)
                return thunk

            def vproj_chunk(c):
                def thunk():
                    ps = psP.tile([128, 512], f32, tag="pj", name="pj")
                    for ci, (k0, kl) in enumerate(CCX):
                        nc.tensor.matmul(ps[:, :260],
                                         x_r[ci][:kl, 128 * c:128 * (c + 1)],
                                         wv_r[ci][:kl, :],
                                         start=(ci == 0), stop=(ci == 4))
                    nc.scalar.activation(v_t[c][:], ps[:, :260], AF.Copy)
                return thunk

            def kv_thunks(c0, w):
                th = [kproj_chunk(c0, w, 0), kproj_chunk(c0, w, 1)]
                th += [vproj_chunk(c0 // 128 + ti) for ti in range(w // 128)]
                return th

            def oproj_thunks(s):
                def piece(m):
                    mc0, mrows = MM[m]

                    def thunk():
                        ps = psP.tile([128, 512], f32, tag="pj", name="pj")
                        for p, (pc0, pl) in enumerate(MM):
                            nc.tensor.matmul(
                                ps[:mrows, :QB],
                                wo_r[p][:pl, mc0:mc0 + mrows],
                                ypr[p][:pl, QB * s:QB * (s + 1)],
                                start=(p == 0), stop=(p == 4))
                        ost = osp.tile([128, QB], f32, tag="ost", name="ost")
                        nc.vector.tensor_copy(ost[:mrows, :], ps[:mrows, :QB])
                        nc.sync.dma_start(
                            yT[mc0:mc0 + mrows, QB * s:QB * (s + 1)],
                            ost[:mrows, :])
                    return thunk
                return [piece(m) for m in range(5)]

            # ---------------- attention --------------------------------
            ycnt = [0]

            def attn_head(s, h):
                seq = _slot_seq(s)
                n = len(seq)
                g = h // 3
                hp, hr = h // 2, 64 * (h % 2)
                yo = QB * (ycnt[0] % 4)
                ycnt[0] += 1
                for sc in range(n // 4):
                    sp = psS.tile([128, 4 * QB], f32, tag="scores",
                                  name="scores")
                    for i in range(4):
                        c = seq[4 * sc + i]
                        nc.tensor.matmul(
                            sp[:, QB * i:QB * (i + 1)],
                            kt_h[g][0:64, 128 * c:128 * (c + 1)],
                            qth[h][0:64, QB * s:QB * (s + 1)],
                            start=True, stop=True)
                    p_b = pw.tile([128, 4 * QB], bf16, tag="p", name="p")
                    nc.scalar.activation(p_b[:], sp[:], AF.Exp, scale=0.125)
                    if sc == n // 4 - 1:
                        nc.vector.tensor_mul(
                            p_b[:], p_b[:], m_b[:, 1024 * s:1024 * (s + 1)])
                    for i in range(4):
                        c = seq[4 * sc + i]
                        nc.tensor.matmul(
                            y_all[:, yo:yo + QB],
                            v_t[c][:, 65 * g:65 * g + 65],
                            p_b[:, QB * i:QB * (i + 1)],
                            start=(4 * sc + i == 0),
                            stop=(4 * sc + i == n - 1))
                recip = dvp.tile([1, QB], f32, tag="recip", name="recip")
                nc.vector.reciprocal(recip[:], y_all[64:65, yo:yo + QB])
                rb_sb = dvp.tile([D, QB], f32, tag="rb", name="rb")
                nc.gpsimd.partition_broadcast(rb_sb[:], recip[:], D)
                nc.vector.tensor_mul(
                    ypr[hp][hr:hr + 64, QB * s:QB * (s + 1)],
                    y_all[0:64, yo:yo + QB], rb_sb[:])

            def interleave(s, proj):
                """Emit slot s's 9 heads, spreading proj thunks between."""
                j = 0
                for i in range(H):
                    attn_head(s, i)
                    want = (i + 1) * len(proj) // H
                    while j < want:
                        proj[j]()
                        j += 1

            # ---------------- emission schedule ------------------------
            for t in kv_thunks(768, 512):        # window A: chunks 6..9
                t()
            for m in range(5):                   # q slot 3
                qproj_chunk(768, 256, m)()

            kvC = kv_thunks(256, 512)                # [K-m0, K-m1, V2..V5]
            # slot 2 consumes kvC's kt cols + diag v chunks 4,5 -- those
            # must be emitted before slot 2's heads (Tile deps follow
            # emission order), so they ride in slot 3's stream.
            interleave(3, kv_thunks(1280, 512)       # window B: 10..13
                       + [qproj_chunk(512, 256, m) for m in range(5)]
                       + [kvC[0], kvC[1], kvC[4], kvC[5]])
            interleave(2, [kvC[2], kvC[3]]           # v chunks 2,3 (slot 1)
                       + [qproj_chunk(0, 256, m) for m in range(5)]   # q slot 0
                       + kv_thunks(0, 256)           # window D: 0..1
                       + kv_thunks(1792, 256))       # window E: 14..15
            interleave(0, [qproj_chunk(256, 256, m) for m in range(5)]  # q slot 1
                       + oproj_thunks(2) + oproj_thunks(3))
            interleave(1, oproj_thunks(0))
            for t in oproj_thunks(1):
                t()

    nc.compile()
    return nc


def _get_program():
    global _PROG
    if _PROG is None:
        _PROG = _build_program()
    return _PROG


def _neox_perm(nheads, swap=False):
    p = []
    for h in range(nheads):
        ev = [64 * h + 2 * j for j in range(32)]
        od = [64 * h + 2 * j + 1 for j in range(32)]
        p += (od + ev) if swap else (ev + od)
    return np.array(p)


_CONSTS = None


def _static_consts():
    """Input-independent per-core constants (tables, masks, key orders)."""
    global _CONSTS
    if _CONSTS is not None:
        return _CONSTS
    invf = THETA ** (-np.arange(32, dtype=np.float64) / 32)

    def tables(pos):
        ang = pos[None, :] * invf[:, None]
        cos, sin = np.cos(ang), np.sin(ang)
        c2 = np.tile(cos, (4, 1)).astype(ml_dtypes.bfloat16)
        s2 = np.tile(np.vstack([-sin, sin]), (2, 1)).astype(np.float32)
        return c2, s2

    per_j = []
    for j in range(2):
        keypos = np.concatenate(
            [np.arange(QB * q, QB * (q + 1)) for q in KEYORDER[j]])
        qsel = keypos[:TQ]          # queries = first 1024 permuted keys
        c2k, s2k = tables(keypos.astype(np.float64))
        masks = np.zeros((128, 16 * QB), np.float32)
        for s in range(4):
            seq = _slot_seq(s)
            qpos = keypos[QB * s:QB * (s + 1)]
            for k in range(4):
                c = seq[-4 + k]
                kpos = keypos[128 * c:128 * (c + 1)]
                masks[:, (4 * s + k) * QB:(4 * s + k + 1) * QB] = (
                    kpos[:, None] <= qpos[None, :]).astype(np.float32)
        per_j.append((keypos, qsel, c2k, s2k,
                      masks.astype(ml_dtypes.bfloat16)))
    _CONSTS = per_j
    return _CONSTS


_PERM = None


def _swap_perm():
    """[128,128] bf16: P[k,m]=1 iff k = pair-swap(m) (+-32 within 64-blocks)."""
    global _PERM
    if _PERM is None:
        P = np.zeros((128, 128), np.float32)
        m = np.arange(128)
        k = np.where(m % 64 < 32, m + 32, m - 32)
        P[k, m] = 1.0
        _PERM = P.astype(ml_dtypes.bfloat16)
    return _PERM


def _host_prep(x, Wq, Wk, Wv, Wo):
    wqT = Wq[_neox_perm(H)].T.astype(ml_dtypes.bfloat16)
    wkT = Wk[_neox_perm(HKV)].T.astype(ml_dtypes.bfloat16)
    woT = Wo.T.astype(ml_dtypes.bfloat16)
    wvT = np.zeros((577, 260), np.float32)
    for g in range(HKV):
        wvT[:C, 65 * g:65 * g + 64] = Wv[64 * g:64 * g + 64].T
        wvT[576, 65 * g + 64] = 1.0
    wvT = wvT.astype(ml_dtypes.bfloat16)

    per_j = _static_consts()
    perm = _swap_perm()
    x = x.astype(ml_dtypes.bfloat16)
    ones = np.ones((1, T), ml_dtypes.bfloat16)
    in_maps = []
    core_meta = []
    for b in range(B):
        xbT = x[b].T
        for j in range(2):
            keypos, qsel, c2k, s2k, masks = per_j[j]
            xkT = np.vstack([xbT[:, keypos], ones])
            in_maps.append({
                "xkT": xkT,
                "wqT": wqT, "wkT": wkT, "wvT": wvT, "woT": woT,
                "c2k": c2k[:32], "s2k": s2k[:64].astype(np.float32),
                "masks": masks, "perm": perm,
            })
            core_meta.append((b, qsel))
    return in_maps, core_meta


def kernel(x, Wq, Wk, Wv, Wo):
    x = np.asarray(x, np.float32)
    Wq = np.asarray(Wq, np.float32)
    Wk = np.asarray(Wk, np.float32)
    Wv = np.asarray(Wv, np.float32)
    Wo = np.asarray(Wo, np.float32)

    from concourse.bass_utils import run_bass_kernel_spmd

    nc = _get_program()
    in_maps, core_meta = _host_prep(x, Wq, Wk, Wv, Wo)
    res = run_bass_kernel_spmd(nc, in_maps, list(range(8)))

    out = np.empty((B, T, C), np.float32)
    for core, (b, qsel) in enumerate(core_meta):
        out[b, qsel, :] = res.results[core]["yT"].T
    return out


# revision 23
# speedup vs baseline: 1.1113x; 1.0279x over previous
"""Trainium2 Bass kernel for CausalSelfAttention (RoPE + GQA), 8-core SPMD.

Sharding: 8 cores = 4 batches x 2 query-halves. Each core owns four
query-256-blocks paired {i, 7-i} so causal work is balanced. Keys are
PERMUTED per core: block order = [own q-blocks (desc causal depth), then
remaining blocks ascending]. Slot s consumes the static key-chunk range
[2s, 2s+PAD_s); its diagonal chunks 2s..2s+1 are emitted last so one bf16
mask multiply per (head, slot) covers diag+pad. The first 1024 key columns
ARE the core's queries, so K rope tables double as Q tables. Every core
runs an identical instruction stream; all variation is input data.

Single fused pipeline per core (vs. baseline's 3 serial phases):
  KV windows are processed in the order [768:1280), [1280:1792),
  [256:768), [0:256), [1792:2048) so attention slot 3 (key chunks 6-9)
  unblocks after ONE window; Q projections are emitted per-slot. RoPE
  needs q/k projected with pair-swapped weights too -- instead of a second
  full projection (5 matmuls), the PSUM is staged to SBUF as bf16 once
  (Act copy, idle engine) and pair-swapped by a single 128x128
  permutation matmul (1 matmul). Post-projection tensors (kt/qth/v/p/ypr,
  tables, masks) are bf16: matmuls stay 1 cycle/row at any width and the
  mask multiply + rope adds hit the DVE 4x mode. Attention per (head,
  slot): S = K^T.T@Q^T with keys on partitions, exp on ScalarE
  (PSUM->bf16, scale=1/8), one bf16 mask multiply, P.V as bf16 matmul
  with a ones-augmented V column yielding the softmax denominator free,
  reciprocal + gpsimd partition-broadcast divide. Attention heads are
  interleaved with projection/output-projection chunks in emission order
  to keep PE busy while ScalarE chews exps; out-projection is emitted in
  256-column slot pieces so only the last slot's piece trails the
  attention stream. DMAs are issued in need-order; x is loaded once and
  stays in SBUF.
"""
import sys

sys.path.insert(0, "/opt/trn_rl_repo")

import numpy as np
import ml_dtypes

B, T, C = 4, 2048, 576
H, HKV, D = 9, 3, 64
THETA = 10000.0
QB = 256                      # query block
TQ = 1024                     # queries per core
SLOT_PAD = [16, 12, 8, 4]     # padded key-chunk counts per slot
QBLOCKS = [[7, 5, 2, 0], [6, 4, 3, 1]]   # q-256-block ids per half j
KEYORDER = [[7, 5, 2, 0, 1, 3, 4, 6], [6, 4, 3, 1, 0, 2, 5, 7]]
CCX = [(0, 128), (128, 128), (256, 128), (384, 128), (512, 65)]   # x chunks (577 rows incl ones)
CCQ = [(0, 128), (128, 128), (256, 128), (384, 128), (512, 64)]   # 576-row chunks
MM = [(0, 128), (128, 128), (256, 128), (384, 128), (512, 64)]    # output-dim chunks of 576
# kv-window processing order (col0, width): slot3's chunks 6..9 first
KVWINS = [(768, 512), (1280, 512), (256, 512), (0, 256), (1792, 256)]
# q-window processing order (col0, width): slot 3, slot 2, slots 0+1
QWINS = [(768, 256), (512, 256), (0, 512)]


def _slot_seq(s):
    """Key-chunk emission order for slot s: fulls, then the two diag chunks."""
    return list(range(2 * s + 2, 2 * s + SLOT_PAD[s])) + [2 * s, 2 * s + 1]


_PROG = None


def _rne12(x):
    """Round fp32 to f32r (RNE, drop 12 mantissa bits) -- matches TRN2."""
    b = np.ascontiguousarray(x, np.float32).view(np.uint32).astype(np.uint64)
    lsb = (b >> np.uint64(12)) & np.uint64(1)
    r = (b + np.uint64(2047) + lsb) >> np.uint64(12) << np.uint64(12)
    return (r & np.uint64(0xFFFFFFFF)).astype(np.uint32).view(np.float32)


def _build_program():
    import concourse.bacc as bacc
    import concourse.mybir as mybir
    import concourse.tile as tile

    dt = mybir.dt
    f32, f32r, bf16 = dt.float32, dt.float32r, dt.bfloat16
    AF = mybir.ActivationFunctionType

    nc = bacc.Bacc("TRN2", target_bir_lowering=False, debug=False, num_devices=8)

    def inp(name, shape, d=f32):
        return nc.declare_dram_parameter(name, shape, d, isOutput=False)

    # hbm params hold the 5 contraction chunks side-by-side on 128
    # partitions so each loads in ONE dma (hwdge pays ~650ns per transfer)
    xkT = inp("xkT", [128, 10240], bf16)     # x: groups A|BE|CD x 5 chunks
    wqT = inp("wqT", [128, 5 * C], bf16)
    wkT = inp("wkT", [128, 5 * HKV * D], bf16)
    wvT = inp("wvT", [128, 5 * 260], bf16)
    woT = inp("woT", [128, 5 * C], bf16)
    c2k = inp("c2k", [32, T], bf16)
    s2k = inp("s2k", [64, T], f32)
    masksp = inp("masks", [128, 16 * QB], bf16)
    permp = inp("perm", [128, 128], bf16)
    yT = nc.declare_dram_parameter("yT", [C, TQ], f32, isOutput=True)

    with tile.TileContext(nc) as tc:
        with (
            tc.tile_pool(name="const", bufs=1) as cp,
            tc.tile_pool(name="rope", bufs=2) as rp,
            tc.tile_pool(name="sbst", bufs=2) as sbp,
            tc.tile_pool(name="pwork", bufs=3) as pw,
            tc.tile_pool(name="ost", bufs=2) as osp,
            tc.tile_pool(name="dvw", bufs=2) as dvp,
            tc.tile_pool(name="psP", bufs=2, space="PSUM") as psP,
            tc.tile_pool(name="psS", bufs=2, space="PSUM") as psS,
            tc.tile_pool(name="psY", bufs=1, space="PSUM") as psYp,
        ):
            # ---------------- persistent SBUF tiles --------------------
            x_all = cp.tile([128, 10240], bf16, tag="xall", name="xall")
            kt_h = [cp.tile([64, T], bf16, tag=f"kt{g}", name=f"kt{g}")
                    for g in range(HKV)]
            qth = [cp.tile([64, TQ], bf16, tag=f"qth{h}", name=f"qth{h}")
                   for h in range(H)]
            v_t = [cp.tile([128, 260], bf16, tag=f"v{c}", name=f"v{c}")
                   for c in range(16)]
            ypr = [cp.tile([128, TQ], bf16, tag=f"ypr{p}", name=f"ypr{p}")
                   for p in range(5)]
            wq_a = cp.tile([128, 5 * C], bf16, tag="wqa", name="wqa")
            wk_a = cp.tile([128, 5 * HKV * D], bf16, tag="wka", name="wka")
            wv_a = cp.tile([128, 5 * 260], bf16, tag="wva", name="wva")
            wo_a = cp.tile([128, 5 * C], bf16, tag="woa", name="woa")
            m_b = cp.tile([128, 16 * QB], bf16, tag="masks", name="masks")
            c2k_t = cp.tile([128, T], bf16, tag="c2k", name="c2k")
            s2k_t = cp.tile([128, T], f32, tag="s2k", name="s2k")
            perm_t = cp.tile([128, 128], bf16, tag="perm", name="perm")
            y_all = psYp.tile([65, 1024], f32, tag="yall", name="yall")

            # ---------------- DMAs in need-order -----------------------
            # x_all column layout: [A: 5ch x 512][BE: 5ch x 768][CD: 5ch x 768]
            XGRP = [(768, 512, 0), (1280, 768, 2560), (0, 768, 6400)]

            def xap(ci, c0):
                for gc0, gw, base in XGRP:
                    if gc0 <= c0 < gc0 + gw:
                        return base + gw * ci + (c0 - gc0)
                raise AssertionError((ci, c0))

            nc.sync.dma_start(wk_a[:], wkT[:])
            nc.sync.dma_start(x_all[:, 0:2560], xkT[:, 0:2560])     # group A
            nc.sync.dma_start(c2k_t[:32, :], c2k[:])
            nc.sync.dma_start(s2k_t[:64, :], s2k[:])
            nc.sync.dma_start(perm_t[:], permp[:])
            # tables repeat with period 32 (cos) / 64 (sin): expand on-chip
            nc.vector.tensor_copy(c2k_t[32:64, :], c2k_t[0:32, :])
            nc.vector.tensor_copy(c2k_t[64:128, :], c2k_t[0:64, :])
            nc.vector.tensor_copy(s2k_t[64:128, :], s2k_t[0:64, :])
            nc.sync.dma_start(wv_a[:], wvT[:])
            nc.sync.dma_start(wq_a[:], wqT[:])
            nc.sync.dma_start(x_all[:, 2560:6400], xkT[:, 2560:6400])   # BE
            nc.sync.dma_start(x_all[:, 6400:10240], xkT[:, 6400:10240])  # CD
            nc.sync.dma_start(m_b[:], masksp[:])
            nc.sync.dma_start(wo_a[:], woT[:])

            # ---------------- rope combine -----------------------------
            def rope(ps, sb_t, mrows, c0, w, dsts):
                """dsts[bi][:, c0:c0+w] = sb*cos + perm(sb)*sin tables."""
                pw_ = psP.tile([128, 512], f32, tag="pj", name="pj")
                nc.tensor.matmul(pw_[:mrows, :w], perm_t[:mrows, :mrows],
                                 sb_t[:mrows, :w], start=True, stop=True)
                t1 = rp.tile([128, 512], bf16, tag="rope1", name="rope1")
                t2 = rp.tile([128, 512], bf16, tag="rope2", name="rope2")
                nc.vector.tensor_mul(t1[:mrows, :w], sb_t[:mrows, :w],
                                     c2k_t[:mrows, c0:c0 + w])
                nc.vector.tensor_mul(t2[:mrows, :w], pw_[:mrows, :w],
                                     s2k_t[:mrows, c0:c0 + w])
                for bi, dt_ in enumerate(dsts):
                    nc.vector.tensor_add(dt_[0:64, c0:c0 + w],
                                         t1[64 * bi:64 * bi + 64, :w],
                                         t2[64 * bi:64 * bi + 64, :w])

            # ---------------- projection thunks ------------------------
            def kproj_chunk(c0, w, mi):
                mc0, mrows = (0, 128) if mi == 0 else (128, 64)
                dsts = [kt_h[0], kt_h[1]] if mi == 0 else [kt_h[2]]

                def thunk():
                    ps = psP.tile([128, 512], f32, tag="pj", name="pj")
                    for ci, (k0, kl) in enumerate(CCQ):
                        wc = 192 * ci + mc0
                        nc.tensor.matmul(ps[:mrows, :w],
                                         wk_a[:kl, wc:wc + mrows],
                                         x_all[:kl, xap(ci, c0):xap(ci, c0) + w],
                                         start=(ci == 0), stop=(ci == 4))
                    sb_t = sbp.tile([128, 512], bf16, tag="sb", name="sb")
                    nc.scalar.activation(sb_t[:mrows, :w], ps[:mrows, :w],
                                         AF.Copy)
                    rope(ps, sb_t, mrows, c0, w, dsts)
                return thunk

            def qproj_chunk(c0, w, m):
                mc0, mrows = MM[m]
                dsts = [qth[2 * m], qth[2 * m + 1]] if m < 4 else [qth[8]]

                def thunk():
                    ps = psP.tile([128, 512], f32, tag="pj", name="pj")
                    for ci, (k0, kl) in enumerate(CCQ):
                        wc = 576 * ci + mc0
                        nc.tensor.matmul(ps[:mrows, :w],
                                         wq_a[:kl, wc:wc + mrows],
                                         x_all[:kl, xap(ci, c0):xap(ci, c0) + w],
                                         start=(ci == 0), stop=(ci == 4))
                    sb_t = sbp.tile([128, 512], bf16, tag="sb", name="sb")
                    nc.scalar.activation(sb_t[:mrows, :w], ps[:mrows, :w],
                                         AF.Copy)
                    rope(ps, sb_t, mrows, c0, w, dsts)
                return thunk

            def vproj_chunk(c):
                def thunk():
                    ps = psP.tile([128, 512], f32, tag="pj", name="pj")
                    for ci, (k0, kl) in enumerate(CCX):
                        xc = xap(ci, 128 * c)
                        nc.tensor.matmul(ps[:, :260],
                                         x_all[:kl, xc:xc + 128],
                                         wv_a[:kl, 260 * ci:260 * (ci + 1)],
                                         start=(ci == 0), stop=(ci == 4))
                    nc.scalar.activation(v_t[c][:], ps[:, :260], AF.Copy)
                return thunk

            def kv_thunks(c0, w):
                th = [kproj_chunk(c0, w, 0), kproj_chunk(c0, w, 1)]
                th += [vproj_chunk(c0 // 128 + ti) for ti in range(w // 128)]
                return th

            def oproj_thunks(s):
                def piece(m):
                    mc0, mrows = MM[m]

                    def thunk():
                        ps = psP.tile([128, 512], f32, tag="pj", name="pj")
                        for p, (pc0, pl) in enumerate(MM):
                            nc.tensor.matmul(
                                ps[:mrows, :QB],
                                wo_a[:pl, 576 * p + mc0:576 * p + mc0 + mrows],
                                ypr[p][:pl, QB * s:QB * (s + 1)],
                                start=(p == 0), stop=(p == 4))
                        ost = osp.tile([128, QB], f32, tag="ost", name="ost")
                        nc.vector.tensor_copy(ost[:mrows, :], ps[:mrows, :QB])
                        nc.sync.dma_start(
                            yT[mc0:mc0 + mrows, QB * s:QB * (s + 1)],
                            ost[:mrows, :])
                    return thunk
                return [piece(m) for m in range(5)]

            # ---------------- attention --------------------------------
            ycnt = [0]

            def attn_head(s, h):
                seq = _slot_seq(s)
                n = len(seq)
                g = h // 3
                hp, hr = h // 2, 64 * (h % 2)
                yo = QB * (ycnt[0] % 4)
                ycnt[0] += 1
                for sc in range(n // 4):
                    sp = psS.tile([128, 4 * QB], f32, tag="scores",
                                  name="scores")
                    for i in range(4):
                        c = seq[4 * sc + i]
                        nc.tensor.matmul(
                            sp[:, QB * i:QB * (i + 1)],
                            kt_h[g][0:64, 128 * c:128 * (c + 1)],
                            qth[h][0:64, QB * s:QB * (s + 1)],
                            start=True, stop=True)
                    p_b = pw.tile([128, 4 * QB], bf16, tag="p", name="p")
                    nc.scalar.activation(p_b[:], sp[:], AF.Exp, scale=0.125)
                    if sc == n // 4 - 1:
                        nc.vector.tensor_mul(
                            p_b[:], p_b[:], m_b[:, 1024 * s:1024 * (s + 1)])
                    for i in range(4):
                        c = seq[4 * sc + i]
                        nc.tensor.matmul(
                            y_all[:, yo:yo + QB],
                            v_t[c][:, 65 * g:65 * g + 65],
                            p_b[:, QB * i:QB * (i + 1)],
                            start=(4 * sc + i == 0),
                            stop=(4 * sc + i == n - 1))
                recip = dvp.tile([1, QB], f32, tag="recip", name="recip")
                nc.vector.reciprocal(recip[:], y_all[64:65, yo:yo + QB])
                rb_sb = dvp.tile([D, QB], f32, tag="rb", name="rb")
                nc.gpsimd.partition_broadcast(rb_sb[:], recip[:], D)
                nc.vector.tensor_mul(
                    ypr[hp][hr:hr + 64, QB * s:QB * (s + 1)],
                    y_all[0:64, yo:yo + QB], rb_sb[:])

            def interleave(s, proj):
                """Emit slot s's 9 heads, spreading proj thunks between."""
                j = 0
                for i in range(H):
                    attn_head(s, i)
                    want = (i + 1) * len(proj) // H
                    while j < want:
                        proj[j]()
                        j += 1

            # ---------------- emission schedule ------------------------
            for t in kv_thunks(768, 512):        # window A: chunks 6..9
                t()
            for m in range(5):                   # q slot 3
                qproj_chunk(768, 256, m)()

            kvC = kv_thunks(256, 512)                # [K-m0, K-m1, V2..V5]
            # slot 2 consumes kvC's kt cols + diag v chunks 4,5 -- those
            # must be emitted before slot 2's heads (Tile deps follow
            # emission order), so they ride in slot 3's stream.
            interleave(3, kv_thunks(1280, 512)       # window B: 10..13
                       + [qproj_chunk(512, 256, m) for m in range(5)]
                       + [kvC[0], kvC[1], kvC[4], kvC[5]])
            interleave(2, [kvC[2], kvC[3]]           # v chunks 2,3 (slot 1)
                       + [qproj_chunk(0, 256, m) for m in range(5)]   # q slot 0
                       + kv_thunks(0, 256)           # window D: 0..1
                       + kv_thunks(1792, 256))       # window E: 14..15
            interleave(0, [qproj_chunk(256, 256, m) for m in range(5)]  # q slot 1
                       + oproj_thunks(2) + oproj_thunks(3))
            interleave(1, oproj_thunks(0))
            for t in oproj_thunks(1):
                t()

    nc.compile()
    return nc


def _get_program():
    global _PROG
    if _PROG is None:
        _PROG = _build_program()
    return _PROG


def _neox_perm(nheads, swap=False):
    p = []
    for h in range(nheads):
        ev = [64 * h + 2 * j for j in range(32)]
        od = [64 * h + 2 * j + 1 for j in range(32)]
        p += (od + ev) if swap else (ev + od)
    return np.array(p)


_CONSTS = None


def _static_consts():
    """Input-independent per-core constants (tables, masks, key orders)."""
    global _CONSTS
    if _CONSTS is not None:
        return _CONSTS
    invf = THETA ** (-np.arange(32, dtype=np.float64) / 32)

    def tables(pos):
        ang = pos[None, :] * invf[:, None]
        cos, sin = np.cos(ang), np.sin(ang)
        c2 = np.tile(cos, (4, 1)).astype(ml_dtypes.bfloat16)
        s2 = np.tile(np.vstack([-sin, sin]), (2, 1)).astype(np.float32)
        return c2, s2

    per_j = []
    for j in range(2):
        keypos = np.concatenate(
            [np.arange(QB * q, QB * (q + 1)) for q in KEYORDER[j]])
        qsel = keypos[:TQ]          # queries = first 1024 permuted keys
        c2k, s2k = tables(keypos.astype(np.float64))
        masks = np.zeros((128, 16 * QB), np.float32)
        for s in range(4):
            seq = _slot_seq(s)
            qpos = keypos[QB * s:QB * (s + 1)]
            for k in range(4):
                c = seq[-4 + k]
                kpos = keypos[128 * c:128 * (c + 1)]
                masks[:, (4 * s + k) * QB:(4 * s + k + 1) * QB] = (
                    kpos[:, None] <= qpos[None, :]).astype(np.float32)
        per_j.append((keypos, qsel, c2k, s2k,
                      masks.astype(ml_dtypes.bfloat16)))
    _CONSTS = per_j
    return _CONSTS


_PERM = None


def _swap_perm():
    """[128,128] bf16: P[k,m]=1 iff k = pair-swap(m) (+-32 within 64-blocks)."""
    global _PERM
    if _PERM is None:
        P = np.zeros((128, 128), np.float32)
        m = np.arange(128)
        k = np.where(m % 64 < 32, m + 32, m - 32)
        P[k, m] = 1.0
        _PERM = P.astype(ml_dtypes.bfloat16)
    return _PERM


def _chunk_cat(a, chunks, cols):
    """[rows, cols] -> [128, 5*cols]: chunk ci's rows side by side."""
    out = np.zeros((128, 5 * cols), a.dtype)
    for ci, (k0, kl) in enumerate(chunks):
        blk = a[k0:k0 + kl]
        out[:blk.shape[0], cols * ci:cols * ci + a.shape[1]] = blk
    return out


_XGRP = [(768, 512, 0), (1280, 768, 2560), (0, 768, 6400)]


def _x_relayout(xkT):
    out = np.zeros((128, 10240), xkT.dtype)
    chunks = [(0, 128), (128, 128), (256, 128), (384, 128), (512, 65)]
    for gc0, gw, base in _XGRP:
        for ci, (k0, kl) in enumerate(chunks):
            out[:kl, base + gw * ci:base + gw * (ci + 1)] = \
                xkT[k0:k0 + kl, gc0:gc0 + gw]
    return out


def _host_prep(x, Wq, Wk, Wv, Wo):
    bf = ml_dtypes.bfloat16
    CC5 = [(0, 128), (128, 128), (256, 128), (384, 128), (512, 65)]
    wqT = _chunk_cat(Wq[_neox_perm(H)].T.astype(bf), CC5, C)
    wkT = _chunk_cat(Wk[_neox_perm(HKV)].T.astype(bf), CC5, HKV * D)
    woT = _chunk_cat(Wo.T.astype(bf), CC5, C)
    wvT = np.zeros((577, 260), np.float32)
    for g in range(HKV):
        wvT[:C, 65 * g:65 * g + 64] = Wv[64 * g:64 * g + 64].T
        wvT[576, 65 * g + 64] = 1.0
    wvT = _chunk_cat(wvT.astype(bf), CC5, 260)

    per_j = _static_consts()
    perm = _swap_perm()
    x = x.astype(bf)
    ones = np.ones((1, T), bf)
    in_maps = []
    core_meta = []
    for b in range(B):
        xbT = x[b].T
        for j in range(2):
            keypos, qsel, c2k, s2k, masks = per_j[j]
            xkT = _x_relayout(np.vstack([xbT[:, keypos], ones]))
            in_maps.append({
                "xkT": xkT,
                "wqT": wqT, "wkT": wkT, "wvT": wvT, "woT": woT,
                "c2k": c2k[:32], "s2k": s2k[:64].astype(np.float32),
                "masks": masks, "perm": perm,
            })
            core_meta.append((b, qsel))
    return in_maps, core_meta


def kernel(x, Wq, Wk, Wv, Wo):
    x = np.asarray(x, np.float32)
    Wq = np.asarray(Wq, np.float32)
    Wk = np.asarray(Wk, np.float32)
    Wv = np.asarray(Wv, np.float32)
    Wo = np.asarray(Wo, np.float32)

    from concourse.bass_utils import run_bass_kernel_spmd

    nc = _get_program()
    in_maps, core_meta = _host_prep(x, Wq, Wk, Wv, Wo)
    res = run_bass_kernel_spmd(nc, in_maps, list(range(8)))

    out = np.empty((B, T, C), np.float32)
    for core, (b, qsel) in enumerate(core_meta):
        out[b, qsel, :] = res.results[core]["yT"].T
    return out
